# revision 2
# baseline (speedup 1.0000x reference)
"""Cross-attention kernel for 8 Trainium2 NeuronCores (SPMD) — v2.

Problem: B=4, T_q=T_kv=2048, Q_DIM=1024, KV_DIM=768, H=16, DK=64, fp32.
  q = q_tokens @ Wq.T ; k = kv_tokens @ Wk.T ; v = kv_tokens @ Wv.T
  out = softmax(q k^T / sqrt(DK)) v @ Wo.T

Sharding (8 cores): core c handles batch b=c//2 and head-group hg=c%2
(8 heads, 512 of the 1024 q-dims).  After attention, the pair (2b, 2b+1)
AllGathers the per-head-group attention outputs (one collective per
head-pair chunk, overlapped with the remaining attention work), then
each core runs the output projection against ITS half of the Wo columns.

v2 vs baseline: everything on-device is bf16 (inputs pre-cast on host),
and K/Q/V live entirely in SBUF — projections write their PSUM results
straight into resident bf16 tiles, so the attention loop does zero DMA.
This keeps the PE streaming (no DMA stalls -> full DVFS pstate) and
halves all remaining HBM/collective traffic.  Softmax runs without
max-subtraction; denominators come free from an appended ones-column in
V during the PV matmul (PSUM partition 64).  Score matmuls for the two
heads of a pair run concurrently in the two 64-row halves of the PE
array (row-group tiling).
"""

import numpy as np
import ml_dtypes

import concourse.bacc as bacc
import concourse.mybir as mybir
import concourse.tile as tile
from concourse import bass_utils

N_CORES = 8
P = 128
TQ = 2048
TKV = 2048
CQ = 1024      # q_tokens channels
CKV = 768      # kv_tokens channels
DQ = 512       # per-core head-group q dims (8 heads x 64)
DO = 512       # per-core output channels (half of 1024)
NJ = 4         # 512-wide q blocks in attention
NI = TKV // P  # 16 kv chunks
NHP = DQ // P  # 4 head-pairs
CQ_CH = CQ // P   # 8
CKV_CH = CKV // P  # 6
NCC = 2 * NHP     # 8 dc chunks in the gathered attention output

F32 = mybir.dt.float32
BF16 = mybir.dt.bfloat16
EXP = mybir.ActivationFunctionType.Exp
MUL = mybir.AluOpType.mult

_compiled = None


def _build():
    nc = bacc.Bacc("TRN2", target_bir_lowering=False, debug=False,
                   num_devices=N_CORES)

    xqT = nc.dram_tensor("xqT", [CQ, TQ], BF16, kind="ExternalInput")
    xkvT = nc.dram_tensor("xkvT", [CKV, TKV], BF16, kind="ExternalInput")
    wqT = nc.dram_tensor("wqT", [CQ, DQ], BF16, kind="ExternalInput")
    wkT = nc.dram_tensor("wkT", [CKV, DQ], BF16, kind="ExternalInput")
    wvT = nc.dram_tensor("wvT", [CKV, DQ], BF16, kind="ExternalInput")
    # full-dc Wo slice for this core's output-channel half; dc rows in
    # global order (core-0 head-group rows then core-1 rows)
    woT = nc.dram_tensor("woT", [2 * DQ, DO], BF16, kind="ExternalInput")
    out_ext = nc.dram_tensor("out", [DO, TQ], BF16, kind="ExternalOutput")

    groups = [[2 * b, 2 * b + 1] for b in range(N_CORES // 2)]

    xq_r = xqT.ap().rearrange("(n p) t -> p n t", p=P)
    xkv_r = xkvT.ap().rearrange("(n p) t -> p n t", p=P)

    with tile.TileContext(nc) as tc:
        with (
            tc.tile_pool(name="res", bufs=1) as rpool,
            tc.tile_pool(name="stage", bufs=1) as stpool,
            tc.tile_pool(name="dram", bufs=1, space="DRAM") as dpool,
        ):
            # ---- resident weights + x + K/Q/V (all bf16) ----
            wv_sb = rpool.tile([P, CKV_CH, DQ], BF16, tag="wv")
            wk_sb = rpool.tile([P, CKV_CH, DQ], BF16, tag="wk")
            wq_sb = rpool.tile([P, CQ_CH, DQ], BF16, tag="wq")
            wo_sb = rpool.tile([P, NCC, DO], BF16, tag="wo")
            xkv_sb = rpool.tile([P, CKV_CH, TKV], BF16, tag="xkv")
            xq_sb = rpool.tile([P, CQ_CH, TQ], BF16, tag="xq")
            kT = rpool.tile([P, NHP, TKV], BF16, tag="kT")
            qT = rpool.tile([P, NHP, TQ], BF16, tag="qT")
            # [kv-in-chunk, i, 4 pairs x (64 dims + ones | 64 dims + ones)]
            vON = rpool.tile([P, NI, NHP * 130], BF16, tag="vON")
            warm = rpool.tile([P, P], BF16, tag="warm")

            nc.sync.dma_start(wv_sb[:], wvT.ap().rearrange("(n p) d -> p n d", p=P))
            nc.scalar.dma_start(xkv_sb[:, :, 0:TKV // 2],
                                xkv_r[:, :, 0:TKV // 2])
            nc.sync.dma_start(wk_sb[:], wkT.ap().rearrange("(n p) d -> p n d", p=P))
            nc.scalar.dma_start(xkv_sb[:, :, TKV // 2:TKV],
                                xkv_r[:, :, TKV // 2:TKV])
            nc.sync.dma_start(wq_sb[:], wqT.ap().rearrange("(n p) d -> p n d", p=P))
            nc.scalar.dma_start(xq_sb[:, :, 0:TQ // 2], xq_r[:, :, 0:TQ // 2])
            nc.sync.dma_start(xq_sb[:, :, TQ // 2:TQ], xq_r[:, :, TQ // 2:TQ])
            nc.scalar.dma_start(wo_sb[:], woT.ap().rearrange("(n p) d -> p n d", p=P))

            # ones columns of vON (col 64 of each head's 65-block)
            nc.vector.memset(warm[:], 0.0)
            vON_ones = vON[:].rearrange("p i (r s) -> p i r s", s=65)[:, :, :, 64:65]
            nc.vector.memset(vON_ones, 1.0)

            # ---- internal DRAM for the pairwise AllGather ----
            ag_in = [dpool.tile([P, TQ], BF16, tag=f"agi{h}", name=f"agi{h}")
                     for h in range(NHP - 1)]
            ag_out = [dpool.tile([2, P, TQ], BF16, tag=f"ago{h}",
                                 name=f"ago{h}")
                      for h in range(NHP - 1)]
            ag_in4 = [dpool.tile([P, TQ // 2], BF16, tag=f"agi4{h}",
                                 name=f"agi4{h}")
                      for h in range(2)]
            ag_out4 = [dpool.tile([2, P, TQ // 2], BF16, tag=f"ago4{h}",
                                  name=f"ago4{h}")
                       for h in range(2)]

            # ================= projections (phase A) =================
            with (
                tc.tile_pool(name="ps_v", bufs=2, space="PSUM") as ps_v,
                tc.tile_pool(name="ps_kq", bufs=2, space="PSUM") as ps_kq,
            ):
                # PE warm-up while the first DMAs stream (ramps the clock)
                for w in range(24):
                    pw = ps_v.tile([P, DQ], F32, tag="ps_v", name=f"warm_{w}")
                    nc.tensor.matmul(pw[:, 0:P], warm[:], warm[:],
                                     start=True, stop=True)

                def v_proj(tc_i):
                    pv = ps_v.tile([P, DQ], F32, tag="ps_v", name=f"pv_{tc_i}")
                    ts_ = slice(tc_i * P, (tc_i + 1) * P)
                    for c in range(CKV_CH):
                        nc.tensor.matmul(pv[:], xkv_sb[:, c, ts_],
                                         wv_sb[:, c, :],
                                         start=(c == 0), stop=(c == CKV_CH - 1))
                    dst = vON[:, tc_i, :].rearrange("p (r s) -> p r s", s=65)
                    nc.vector.tensor_copy(
                        dst[:, :, 0:64],
                        pv[:].rearrange("p (r s) -> p r s", s=64))

                def k_proj(hp, kvh):
                    pk = ps_kq.tile([P, 512], F32, tag="ps_kq",
                                    name=f"pk_{hp}_{kvh}")
                    hs = slice(hp * P, (hp + 1) * P)
                    ks_ = slice(kvh * 512, (kvh + 1) * 512)
                    for c in range(CKV_CH):
                        nc.tensor.matmul(pk[:], wk_sb[:, c, hs],
                                         xkv_sb[:, c, ks_],
                                         start=(c == 0), stop=(c == CKV_CH - 1))
                    nc.scalar.copy(kT[:, hp, ks_], pk[:])

                def q_proj(hp, qh):
                    pq = ps_kq.tile([P, 512], F32, tag="ps_kq",
                                    name=f"pq_{hp}_{qh}")
                    hs = slice(hp * P, (hp + 1) * P)
                    qs_ = slice(qh * 512, (qh + 1) * 512)
                    for c in range(CQ_CH):
                        nc.tensor.matmul(pq[:], wq_sb[:, c, hs],
                                         xq_sb[:, c, qs_],
                                         start=(c == 0), stop=(c == CQ_CH - 1))
                    if qh % 2 == 0:
                        nc.scalar.copy(qT[:, hp, qs_], pq[:])
                    else:
                        nc.vector.tensor_copy(qT[:, hp, qs_], pq[:])

                # NOTE: emission order matters — releasing attention's deps
                # early lets the scheduler interleave attention with the
                # projections, which starves the (pacing) Act exp stream.
                # Keep phases strictly sequential.
                for tc_i in range(8):
                    v_proj(tc_i)
                k_proj(0, 0)
                k_proj(0, 1)
                for tc_i in range(8, 16):
                    v_proj(tc_i)
                k_proj(0, 2)
                k_proj(0, 3)
                for qh in range(4):
                    q_proj(0, qh)
                for hp in range(1, NHP):
                    for kvh in range(4):
                        k_proj(hp, kvh)
                    for qh in range(4):
                        q_proj(hp, qh)

            # ========== attention, with per-head-pair AllGather ==========
            with (
                tc.tile_pool(name="ps_s", bufs=2, space="PSUM") as ps_s,
                tc.tile_pool(name="ps_pv", bufs=4, space="PSUM") as ps_pv,
            ):
                for hp in range(NHP):
                    ao = stpool.tile([P, TQ], BF16, tag="ao", bufs=2)
                    vsl_a = slice(hp * 130, hp * 130 + 65)
                    vsl_b = slice(hp * 130 + 65, hp * 130 + 130)
                    for j in range(NJ):
                        js = slice(j * 512, (j + 1) * 512)
                        acc_a = ps_pv.tile([P, 512], F32, tag="pv")
                        acc_b = ps_pv.tile([P, 512], F32, tag="pv")
                        for i in range(NI):
                            isl = slice(i * P, (i + 1) * P)
                            sc = ps_s.tile([P, 1024], F32, tag="sc")
                            nc.tensor.matmul(sc[:, 0:512], kT[0:64, hp, isl],
                                             qT[0:64, hp, js], start=True,
                                             stop=True)
                            nc.tensor.matmul(sc[:, 512:1024],
                                             kT[64:128, hp, isl],
                                             qT[64:128, hp, js], start=True,
                                             stop=True)
                            ex = stpool.tile([P, 1024], BF16, tag="ex", bufs=3)
                            nc.scalar.activation(ex[:], sc[:], EXP, scale=0.125)
                            nc.tensor.matmul(acc_a[0:65, :], vON[:, i, vsl_a],
                                             ex[:, 0:512],
                                             start=(i == 0), stop=(i == NI - 1))
                            nc.tensor.matmul(acc_b[0:65, :], vON[:, i, vsl_b],
                                             ex[:, 512:1024],
                                             start=(i == 0), stop=(i == NI - 1))
                        # evict + normalize: ao[:, js] = acc[0:64] / acc[64]
                        for half, acc in ((0, acc_a), (1, acc_b)):
                            pvst = stpool.tile([P, 512], F32, tag="pvst",
                                               bufs=3,
                                               name=f"pvst_{hp}_{j}_{half}")
                            nc.vector.tensor_copy(pvst[0:65, :], acc[0:65, :])
                            rec = stpool.tile([P, 512], F32, tag="rec", bufs=2)
                            nc.vector.reciprocal(rec[0:1, :], pvst[64:65, :])
                            bc = stpool.tile([P, 512], F32, tag="bc", bufs=2)
                            nc.gpsimd.partition_broadcast(bc[0:64, :],
                                                          rec[0:1, :],
                                                          channels=64)
                            nc.vector.tensor_tensor(
                                ao[half * 64:(half + 1) * 64, js],
                                pvst[0:64, :], bc[0:64, :], op=MUL)
                        # the last head-pair's exchange goes in two halves,
                        # issued as soon as each half of ao is complete
                        if hp == NHP - 1 and j in (1, NJ - 1):
                            hl = j // 2
                            hsl = slice(hl * 1024, (hl + 1) * 1024)
                            nc.sync.dma_start(ag_in4[hl][:], ao[:, hsl])
                            nc.gpsimd.collective_compute(
                                "AllGather", mybir.AluOpType.bypass,
                                replica_groups=groups,
                                ins=[ag_in4[hl].opt()],
                                outs=[ag_out4[hl].opt()])
                    # exchange this head-pair's attention output with the
                    # pair peer while later head-pairs keep computing
                    if hp < NHP - 1:
                        nc.sync.dma_start(ag_in[hp][:], ao[:])
                        nc.gpsimd.collective_compute(
                            "AllGather", mybir.AluOpType.bypass,
                            replica_groups=groups,
                            ins=[ag_in[hp].opt()], outs=[ag_out[hp].opt()])

                # ===== output projection (my half of the Wo columns) =====
                # po tiles reuse the sc pool: slots free as attention drains
                for j in range(NJ):
                    js = slice(j * 512, (j + 1) * 512)
                    rhs = []
                    for n in range(NCC):
                        g, hp = n // NHP, n % NHP
                        aog = stpool.tile([P, 512], BF16, tag="aog",
                                          bufs=16, name=f"aog_{j}_{g}_{hp}")
                        eng = nc.sync if n % 2 == 0 else nc.scalar
                        if hp < NHP - 1:
                            eng.dma_start(aog[:], ag_out[hp][g, :, js])
                        else:
                            eng.dma_start(
                                aog[:],
                                ag_out4[j // 2][g, :,
                                                (j % 2) * 512:(j % 2 + 1) * 512])
                        rhs.append(aog)
                    for do in range(DO // P):
                        po = ps_s.tile([P, 512], F32, tag="sc",
                                       name=f"po_{j}_{do}")
                        for n in range(NCC):
                            nc.tensor.matmul(
                                po[:], wo_sb[:, n, do * P:(do + 1) * P],
                                rhs[n][:],
                                start=(n == 0), stop=(n == NCC - 1))
                        ost = stpool.tile([P, 512], BF16, tag="ost", bufs=3)
                        if do % 2 == 0:
                            nc.vector.tensor_copy(ost[:], po[:])
                        else:
                            nc.scalar.copy(ost[:], po[:])
                        nc.sync.dma_start(out_ext[do * P:(do + 1) * P, js],
                                          ost[:])

    nc.compile()
    return nc


def make_in_maps(q_tokens, kv_tokens, Wq, Wk, Wv, Wo):
    bf = ml_dtypes.bfloat16
    q_tokens = np.asarray(q_tokens, np.float32)
    kv_tokens = np.asarray(kv_tokens, np.float32)
    Wq = np.asarray(Wq, np.float32)
    Wk = np.asarray(Wk, np.float32)
    Wv = np.asarray(Wv, np.float32)
    Wo = np.asarray(Wo, np.float32)
    in_maps = []
    for c in range(N_CORES):
        b, hg = c // 2, c % 2
        sl = slice(hg * DQ, (hg + 1) * DQ)
        osl = slice(hg * DO, (hg + 1) * DO)
        in_maps.append({
            "xqT": np.ascontiguousarray(q_tokens[b].T).astype(bf),
            "xkvT": np.ascontiguousarray(kv_tokens[b].T).astype(bf),
            "wqT": np.ascontiguousarray(Wq[sl, :].T).astype(bf),
            "wkT": np.ascontiguousarray(Wk[sl, :].T).astype(bf),
            "wvT": np.ascontiguousarray(Wv[sl, :].T).astype(bf),
            # [dc, oc-half] with dc rows in global (gathered) order
            "woT": np.ascontiguousarray(Wo[osl, :].T).astype(bf),
        })
    return in_maps


def kernel(q_tokens, kv_tokens, Wq, Wk, Wv, Wo):
    global _compiled
    if _compiled is None:
        _compiled = _build()
    nc = _compiled

    in_maps = make_in_maps(q_tokens, kv_tokens, Wq, Wk, Wv, Wo)
    res = bass_utils.run_bass_kernel_spmd(nc, in_maps,
                                          core_ids=list(range(N_CORES)))
    B = 4
    out = np.empty((B, TQ, 2 * DO), np.float32)
    for c in range(N_CORES):
        b, hg = c // 2, c % 2
        out[b, :, hg * DO:(hg + 1) * DO] = \
            np.asarray(res.results[c]["out"]).astype(np.float32).T
    return out


# revision 4
# speedup vs baseline: 1.0078x; 1.0078x over previous
"""Cross-attention kernel for 8 Trainium2 NeuronCores (SPMD) — v2.

Problem: B=4, T_q=T_kv=2048, Q_DIM=1024, KV_DIM=768, H=16, DK=64, fp32.
  q = q_tokens @ Wq.T ; k = kv_tokens @ Wk.T ; v = kv_tokens @ Wv.T
  out = softmax(q k^T / sqrt(DK)) v @ Wo.T

Sharding (8 cores): core c handles batch b=c//2 and head-group hg=c%2
(8 heads, 512 of the 1024 q-dims).  After attention, the pair (2b, 2b+1)
AllGathers the per-head-group attention outputs (one collective per
head-pair chunk, overlapped with the remaining attention work), then
each core runs the output projection against ITS half of the Wo columns.

v2 vs baseline: everything on-device is bf16 (inputs pre-cast on host),
and K/Q/V live entirely in SBUF — projections write their PSUM results
straight into resident bf16 tiles, so the attention loop does zero DMA.
This keeps the PE streaming (no DMA stalls -> full DVFS pstate) and
halves all remaining HBM/collective traffic.  Softmax runs without
max-subtraction; denominators come free from an appended ones-column in
V during the PV matmul (PSUM partition 64).  Score matmuls for the two
heads of a pair run concurrently in the two 64-row halves of the PE
array (row-group tiling).
"""

import numpy as np
import ml_dtypes

import concourse.bacc as bacc
import concourse.mybir as mybir
import concourse.tile as tile
from concourse import bass_utils

N_CORES = 8
P = 128
TQ = 2048
TKV = 2048
CQ = 1024      # q_tokens channels
CKV = 768      # kv_tokens channels
DQ = 512       # per-core head-group q dims (8 heads x 64)
DO = 512       # per-core output channels (half of 1024)
NJ = 4         # 512-wide q blocks in attention
NI = TKV // P  # 16 kv chunks
NHP = DQ // P  # 4 head-pairs
CQ_CH = CQ // P   # 8
CKV_CH = CKV // P  # 6
NCC = 2 * NHP     # 8 dc chunks in the gathered attention output

F32 = mybir.dt.float32
BF16 = mybir.dt.bfloat16
EXP = mybir.ActivationFunctionType.Exp
MUL = mybir.AluOpType.mult

_compiled = None


def _build():
    nc = bacc.Bacc("TRN2", target_bir_lowering=False, debug=False,
                   num_devices=N_CORES)

    xqT = nc.dram_tensor("xqT", [CQ, TQ], BF16, kind="ExternalInput")
    xkvT = nc.dram_tensor("xkvT", [CKV, TKV], BF16, kind="ExternalInput")
    wqT = nc.dram_tensor("wqT", [CQ, DQ], BF16, kind="ExternalInput")
    wkT = nc.dram_tensor("wkT", [CKV, DQ], BF16, kind="ExternalInput")
    wvT = nc.dram_tensor("wvT", [CKV, DQ], BF16, kind="ExternalInput")
    # full-dc Wo slice for this core's output-channel half; dc rows in
    # global order (core-0 head-group rows then core-1 rows)
    woT = nc.dram_tensor("woT", [2 * DQ, DO], BF16, kind="ExternalInput")
    out_ext = nc.dram_tensor("out", [DO, TQ], BF16, kind="ExternalOutput")

    groups = [[2 * b, 2 * b + 1] for b in range(N_CORES // 2)]

    xq_r = xqT.ap().rearrange("(n p) t -> p n t", p=P)
    xkv_r = xkvT.ap().rearrange("(n p) t -> p n t", p=P)

    with tile.TileContext(nc) as tc:
        with (
            tc.tile_pool(name="res", bufs=1) as rpool,
            tc.tile_pool(name="stage", bufs=1) as stpool,
            tc.tile_pool(name="dram", bufs=1, space="DRAM") as dpool,
        ):
            # ---- resident weights + x + K/Q/V (all bf16) ----
            wv_sb = rpool.tile([P, CKV_CH, DQ], BF16, tag="wv")
            wk_sb = rpool.tile([P, CKV_CH, DQ], BF16, tag="wk")
            wq_sb = rpool.tile([P, CQ_CH, DQ], BF16, tag="wq")
            wo_sb = rpool.tile([P, NCC, DO], BF16, tag="wo")
            xkv_sb = rpool.tile([P, CKV_CH, TKV], BF16, tag="xkv")
            xq_sb = rpool.tile([P, CQ_CH, TQ], BF16, tag="xq")
            kT = rpool.tile([P, NHP, TKV], BF16, tag="kT")
            qT = rpool.tile([P, NHP, TQ], BF16, tag="qT")
            # [kv-in-chunk, i, 4 pairs x (64 dims + ones | 64 dims + ones)]
            vON = rpool.tile([P, NI, NHP * 130], BF16, tag="vON")
            warm = rpool.tile([P, P], BF16, tag="warm")

            nc.sync.dma_start(wv_sb[:], wvT.ap().rearrange("(n p) d -> p n d", p=P))
            nc.scalar.dma_start(xkv_sb[:, :, 0:TKV // 2],
                                xkv_r[:, :, 0:TKV // 2])
            nc.sync.dma_start(wk_sb[:], wkT.ap().rearrange("(n p) d -> p n d", p=P))
            nc.scalar.dma_start(xkv_sb[:, :, TKV // 2:TKV],
                                xkv_r[:, :, TKV // 2:TKV])
            nc.sync.dma_start(wq_sb[:], wqT.ap().rearrange("(n p) d -> p n d", p=P))
            nc.scalar.dma_start(xq_sb[:, :, 0:TQ // 2], xq_r[:, :, 0:TQ // 2])
            nc.sync.dma_start(xq_sb[:, :, TQ // 2:TQ], xq_r[:, :, TQ // 2:TQ])
            nc.scalar.dma_start(wo_sb[:], woT.ap().rearrange("(n p) d -> p n d", p=P))

            # ones columns of vON (col 64 of each head's 65-block)
            nc.vector.memset(warm[:], 0.0)
            vON_ones = vON[:].rearrange("p i (r s) -> p i r s", s=65)[:, :, :, 64:65]
            nc.vector.memset(vON_ones, 1.0)

            # ---- internal DRAM for the pairwise AllGather ----
            ag_in = [dpool.tile([P, TQ], BF16, tag=f"agi{h}", name=f"agi{h}")
                     for h in range(NHP - 1)]
            ag_out = [dpool.tile([2, P, TQ], BF16, tag=f"ago{h}",
                                 name=f"ago{h}")
                      for h in range(NHP - 1)]
            ag_in4 = [dpool.tile([P, TQ // 2], BF16, tag=f"agi4{h}",
                                 name=f"agi4{h}")
                      for h in range(2)]
            ag_out4 = [dpool.tile([2, P, TQ // 2], BF16, tag=f"ago4{h}",
                                  name=f"ago4{h}")
                       for h in range(2)]

            # ================= projections (phase A) =================
            with (
                tc.tile_pool(name="ps_v", bufs=2, space="PSUM") as ps_v,
                tc.tile_pool(name="ps_kq", bufs=2, space="PSUM") as ps_kq,
            ):
                # PE warm-up while the first DMAs stream (ramps the clock)
                for w in range(24):
                    pw = ps_v.tile([P, DQ], F32, tag="ps_v", name=f"warm_{w}")
                    nc.tensor.matmul(pw[:, 0:P], warm[:], warm[:],
                                     start=True, stop=True)

                def v_proj(tc_i):
                    pv = ps_v.tile([P, DQ], F32, tag="ps_v", name=f"pv_{tc_i}")
                    ts_ = slice(tc_i * P, (tc_i + 1) * P)
                    for c in range(CKV_CH):
                        nc.tensor.matmul(pv[:], xkv_sb[:, c, ts_],
                                         wv_sb[:, c, :],
                                         start=(c == 0), stop=(c == CKV_CH - 1))
                    dst = vON[:, tc_i, :].rearrange("p (r s) -> p r s", s=65)
                    nc.vector.tensor_copy(
                        dst[:, :, 0:64],
                        pv[:].rearrange("p (r s) -> p r s", s=64))

                def k_proj(hp, kvh):
                    pk = ps_kq.tile([P, 512], F32, tag="ps_kq",
                                    name=f"pk_{hp}_{kvh}")
                    hs = slice(hp * P, (hp + 1) * P)
                    ks_ = slice(kvh * 512, (kvh + 1) * 512)
                    for c in range(CKV_CH):
                        nc.tensor.matmul(pk[:], wk_sb[:, c, hs],
                                         xkv_sb[:, c, ks_],
                                         start=(c == 0), stop=(c == CKV_CH - 1))
                    nc.scalar.copy(kT[:, hp, ks_], pk[:])

                def q_proj(hp, qh):
                    pq = ps_kq.tile([P, 512], F32, tag="ps_kq",
                                    name=f"pq_{hp}_{qh}")
                    hs = slice(hp * P, (hp + 1) * P)
                    qs_ = slice(qh * 512, (qh + 1) * 512)
                    for c in range(CQ_CH):
                        nc.tensor.matmul(pq[:], wq_sb[:, c, hs],
                                         xq_sb[:, c, qs_],
                                         start=(c == 0), stop=(c == CQ_CH - 1))
                    if qh % 2 == 0:
                        nc.scalar.copy(qT[:, hp, qs_], pq[:])
                    else:
                        nc.vector.tensor_copy(qT[:, hp, qs_], pq[:])

                # NOTE: emission order matters — releasing attention's deps
                # early lets the scheduler interleave attention with the
                # projections, which starves the (pacing) Act exp stream.
                # Keep phases strictly sequential.
                for tc_i in range(8):
                    v_proj(tc_i)
                k_proj(0, 0)
                k_proj(0, 1)
                for tc_i in range(8, 16):
                    v_proj(tc_i)
                k_proj(0, 2)
                k_proj(0, 3)
                for qh in range(4):
                    q_proj(0, qh)
                for hp in range(1, NHP):
                    for kvh in range(4):
                        k_proj(hp, kvh)
                    for qh in range(4):
                        q_proj(hp, qh)

            # ========== attention, with per-head-pair AllGather ==========
            with (
                tc.tile_pool(name="ps_s", bufs=2, space="PSUM") as ps_s,
                tc.tile_pool(name="ps_pv", bufs=4, space="PSUM") as ps_pv,
            ):
                for hp in range(NHP):
                    ao = stpool.tile([P, TQ], BF16, tag="ao", bufs=2)
                    vsl_a = slice(hp * 130, hp * 130 + 65)
                    vsl_b = slice(hp * 130 + 65, hp * 130 + 130)
                    for j in range(NJ):
                        js = slice(j * 512, (j + 1) * 512)
                        acc_a = ps_pv.tile([P, 512], F32, tag="pv")
                        acc_b = ps_pv.tile([P, 512], F32, tag="pv")
                        for i in range(NI):
                            isl = slice(i * P, (i + 1) * P)
                            sc = ps_s.tile([P, 1024], F32, tag="sc")
                            nc.tensor.matmul(sc[:, 0:512], kT[0:64, hp, isl],
                                             qT[0:64, hp, js], start=True,
                                             stop=True)
                            nc.tensor.matmul(sc[:, 512:1024],
                                             kT[64:128, hp, isl],
                                             qT[64:128, hp, js], start=True,
                                             stop=True)
                            ex = stpool.tile([P, 1024], BF16, tag="ex", bufs=4)
                            nc.scalar.activation(ex[:], sc[:], EXP, scale=0.125)
                            nc.tensor.matmul(acc_a[0:65, :], vON[:, i, vsl_a],
                                             ex[:, 0:512],
                                             start=(i == 0), stop=(i == NI - 1))
                            nc.tensor.matmul(acc_b[0:65, :], vON[:, i, vsl_b],
                                             ex[:, 512:1024],
                                             start=(i == 0), stop=(i == NI - 1))
                        # evict + normalize: ao[:, js] = acc[0:64] / acc[64]
                        for half, acc in ((0, acc_a), (1, acc_b)):
                            pvst = stpool.tile([P, 512], F32, tag="pvst",
                                               bufs=3,
                                               name=f"pvst_{hp}_{j}_{half}")
                            nc.vector.tensor_copy(pvst[0:65, :], acc[0:65, :])
                            rec = stpool.tile([P, 512], F32, tag="rec", bufs=2)
                            nc.vector.reciprocal(rec[0:1, :], pvst[64:65, :])
                            bc = stpool.tile([P, 512], F32, tag="bc", bufs=2)
                            nc.gpsimd.partition_broadcast(bc[0:64, :],
                                                          rec[0:1, :],
                                                          channels=64)
                            nc.vector.tensor_tensor(
                                ao[half * 64:(half + 1) * 64, js],
                                pvst[0:64, :], bc[0:64, :], op=MUL)
                        # the last head-pair's exchange goes in two halves,
                        # issued as soon as each half of ao is complete
                        if hp == NHP - 1 and j in (1, NJ - 1):
                            hl = j // 2
                            hsl = slice(hl * 1024, (hl + 1) * 1024)
                            nc.sync.dma_start(ag_in4[hl][:], ao[:, hsl])
                            nc.gpsimd.collective_compute(
                                "AllGather", mybir.AluOpType.bypass,
                                replica_groups=groups,
                                ins=[ag_in4[hl].opt()],
                                outs=[ag_out4[hl].opt()])
                    # exchange this head-pair's attention output with the
                    # pair peer while later head-pairs keep computing
                    if hp < NHP - 1:
                        nc.sync.dma_start(ag_in[hp][:], ao[:])
                        nc.gpsimd.collective_compute(
                            "AllGather", mybir.AluOpType.bypass,
                            replica_groups=groups,
                            ins=[ag_in[hp].opt()], outs=[ag_out[hp].opt()])

                # ===== output projection (my half of the Wo columns) =====
                # po tiles reuse the sc pool: slots free as attention drains
                for j in range(NJ):
                    js = slice(j * 512, (j + 1) * 512)
                    rhs = []
                    for n in range(NCC):
                        g, hp = n // NHP, n % NHP
                        aog = stpool.tile([P, 512], BF16, tag="aog",
                                          bufs=16, name=f"aog_{j}_{g}_{hp}")
                        eng = nc.sync if n % 2 == 0 else nc.scalar
                        if hp < NHP - 1:
                            eng.dma_start(aog[:], ag_out[hp][g, :, js])
                        else:
                            eng.dma_start(
                                aog[:],
                                ag_out4[j // 2][g, :,
                                                (j % 2) * 512:(j % 2 + 1) * 512])
                        rhs.append(aog)
                    for do in range(DO // P):
                        po = ps_pv.tile([P, 512], F32, tag="pv",
                                        name=f"po_{j}_{do}")
                        for n in range(NCC):
                            nc.tensor.matmul(
                                po[:], wo_sb[:, n, do * P:(do + 1) * P],
                                rhs[n][:],
                                start=(n == 0), stop=(n == NCC - 1))
                        ost = stpool.tile([P, 512], BF16, tag="ost", bufs=3)
                        if do % 2 == 0:
                            nc.vector.tensor_copy(ost[:], po[:])
                        else:
                            nc.scalar.copy(ost[:], po[:])
                        nc.sync.dma_start(out_ext[do * P:(do + 1) * P, js],
                                          ost[:])

    nc.compile()
    return nc


def make_in_maps(q_tokens, kv_tokens, Wq, Wk, Wv, Wo):
    bf = ml_dtypes.bfloat16
    q_tokens = np.asarray(q_tokens, np.float32)
    kv_tokens = np.asarray(kv_tokens, np.float32)
    Wq = np.asarray(Wq, np.float32)
    Wk = np.asarray(Wk, np.float32)
    Wv = np.asarray(Wv, np.float32)
    Wo = np.asarray(Wo, np.float32)
    in_maps = []
    for c in range(N_CORES):
        b, hg = c // 2, c % 2
        sl = slice(hg * DQ, (hg + 1) * DQ)
        osl = slice(hg * DO, (hg + 1) * DO)
        in_maps.append({
            "xqT": np.ascontiguousarray(q_tokens[b].T).astype(bf),
            "xkvT": np.ascontiguousarray(kv_tokens[b].T).astype(bf),
            "wqT": np.ascontiguousarray(Wq[sl, :].T).astype(bf),
            "wkT": np.ascontiguousarray(Wk[sl, :].T).astype(bf),
            "wvT": np.ascontiguousarray(Wv[sl, :].T).astype(bf),
            # [dc, oc-half] with dc rows in global (gathered) order
            "woT": np.ascontiguousarray(Wo[osl, :].T).astype(bf),
        })
    return in_maps


def kernel(q_tokens, kv_tokens, Wq, Wk, Wv, Wo):
    global _compiled
    if _compiled is None:
        _compiled = _build()
    nc = _compiled

    in_maps = make_in_maps(q_tokens, kv_tokens, Wq, Wk, Wv, Wo)
    res = bass_utils.run_bass_kernel_spmd(nc, in_maps,
                                          core_ids=list(range(N_CORES)))
    B = 4
    out = np.empty((B, TQ, 2 * DO), np.float32)
    for c in range(N_CORES):
        b, hg = c // 2, c % 2
        out[b, :, hg * DO:(hg + 1) * DO] = \
            np.asarray(res.results[c]["out"]).astype(np.float32).T
    return out
